# revision 1
# baseline (speedup 1.0000x reference)
"""Trainium2 Bass kernel for nn_BinaryTokenClassificationModel (segment_reduce).

Math: the reference pools token embeddings into word embeddings (mean over
contiguous runs of equal word ids), then computes
    logits[b,s,t] = src_pooled[b,s] @ w_src + tgt_pooled[b,t] @ w_tgt + b.
Because the classifier is linear, pooling and projection commute:
    src_proj[w] = sum_t A[w,t] * (tok_h[t] @ w_src)     (A = 1/count-weighted
    tgt_proj[w] = sum_t A[w,t] * (tok_h[t] @ w_tgt)      segment membership)
and the output is the outer sum src_proj[s] + tgt_proj[t] + b. Each core:
  1. streams its batch row of tok_h [512, 768] through a fused DVE
     multiply-reduce against the broadcast weight row -> u[t] (per-token scalar)
  2. builds the membership matrix on-device (GpSimd iota + compare against the
     per-token segment index) -- no membership DMA traffic
  3. accumulates  atw_c.T @ broadcast(u_c)  (src chunks) and
     broadcast(u_c).T @ atw_c  (tgt chunks) straight into the [S, T] output
     PSUM tile, which realizes segment-reduce + outer-sum in one matmul/chunk.
Data-parallel over batch: core i handles batch row i. No collectives.
"""

import functools

import numpy as np

import concourse.bacc as bacc
import concourse.mybir as mybir
from concourse.bass_utils import run_bass_kernel_spmd
from concourse.tile import TileContext
from concourse.tile_rust import add_dep_helper

# Problem geometry (hardcoded per spec)
B = 8
L_SRC = 256
L_TGT = 256
L = L_SRC + L_TGT  # 512
H = 768
P = 128            # SBUF partitions / tokens per chunk
NCHUNK = L // P    # 4
N_SRC_CHUNKS = L_SRC // P  # 2
N_CORES = 8
F32 = mybir.dt.float32


# ---------------------------------------------------------------------------
# Host-side segment bookkeeping (exact mirror of reference._pool_words)
# ---------------------------------------------------------------------------

def _segments(combined_wid, attention_mask, n_words):
    """Per-token dense run ids exactly as the reference computes them."""
    valid = (attention_mask > 0) & (combined_wid >= 0)  # [B, L]
    prev_wid = np.concatenate(
        [np.full((combined_wid.shape[0], 1), -2, dtype=combined_wid.dtype),
         combined_wid[:, :-1]], axis=1)
    prev_valid = np.concatenate(
        [np.zeros((valid.shape[0], 1), dtype=bool), valid[:, :-1]], axis=1)
    new_run = valid & ((combined_wid != prev_wid) | (~prev_valid))
    run_id = np.cumsum(new_run.astype(np.int64), axis=1) - 1  # [B, L]
    seg = np.where(valid, run_id, n_words)  # n_words = dummy slot
    return seg, valid


def _seg_weights(seg, valid, n_words):
    """1/max(count,1) weight for each token's segment (0 for invalid)."""
    Bv, Lv = seg.shape
    wgt = np.zeros((Bv, Lv), dtype=np.float32)
    for b in range(Bv):
        counts = np.bincount(seg[b][valid[b]], minlength=Lv + 1).astype(np.float32)
        inv = 1.0 / np.maximum(counts, 1.0)
        wgt[b] = np.where(valid[b] & (seg[b] < n_words), inv[np.minimum(seg[b], Lv)], 0.0)
    return wgt


def _host_forward(tok_h, attention_mask, source_word_ids, target_word_ids, W, b, S, T):
    """Pure numpy forward implementing the same algebra the device runs.

    Used for validation only (test harness); not called by kernel().
    """
    combined = np.concatenate([source_word_ids, target_word_ids], axis=1).astype(np.int64)
    seg, valid = _segments(combined, np.asarray(attention_mask), S + T)
    wgt = _seg_weights(seg, valid, S + T)
    w_src = W[:H, 0].astype(np.float32)
    w_tgt = W[H:2 * H, 0].astype(np.float32)
    out = np.empty((tok_h.shape[0], S, T), dtype=np.float32)
    for bi in range(tok_h.shape[0]):
        u_src = tok_h[bi].astype(np.float32) @ w_src  # [L]
        u_tgt = tok_h[bi].astype(np.float32) @ w_tgt  # [L]
        proj = np.zeros(S + T, dtype=np.float32)
        for t in range(L):
            s = seg[bi, t]
            if s < S:
                proj[s] += wgt[bi, t] * u_src[t]
            elif s < S + T:
                proj[s] += wgt[bi, t] * u_tgt[t]
        out[bi] = proj[:S, None] + proj[None, S:S + T] + float(np.asarray(b).reshape(-1)[0])
    return out


# ---------------------------------------------------------------------------
# Device kernel, fast path (block_ok): src tokens -> word rows [0,S),
# tgt tokens -> word rows [S,S+T)
# ---------------------------------------------------------------------------

def _declare_block_params(nc, S, T):
    MW = 2 * NCHUNK  # meta columns
    return dict(
        tok0=nc.declare_dram_parameter("tok0", [P, H + MW], F32, isOutput=False),
        tok1=nc.declare_dram_parameter("tok1", [P, H], F32, isOutput=False),
        tok2=nc.declare_dram_parameter("tok2", [P, H], F32, isOutput=False),
        tok3=nc.declare_dram_parameter("tok3", [P, H], F32, isOutput=False),
        # wcat = [w_src (H) | w_tgt (H) | bias (1)]
        wcat=nc.declare_dram_parameter("wcat", [1, 2 * H + 1], F32, isOutput=False),
        # iotac[p, w] = w  (constant; GpSimd iota is a slow SW op on HW)
        iotac=nc.declare_dram_parameter("iotac", [P, P], F32, isOutput=False),
        out=nc.declare_dram_parameter("out", [S, T], F32, isOutput=True),
    )


def _emit_block_body(nc, tc, prm, S, T, mm_mode="mat", prod_space="SBUF"):
    """Fast path. DMA layout: chunk 0 and chunk 3 token loads are split in
    half-rows -- chunk 0 so the (tiny, pipeline-gating) wcat transfer can slip
    into the DMA stream between the halves, chunk 3 so the tail reduce after
    the last byte lands is half-length. meta rides as extra columns packed
    into the first token piece (no DMA of its own)."""
    MW = 2 * NCHUNK
    tok0, tok1, tok2, tok3, wcat, iotac, out = (
        prm["tok0"], prm["tok1"], prm["tok2"], prm["tok3"],
        prm["wcat"], prm["iotac"], prm["out"])
    if True:
        with (
            tc.tile_pool(name="const", bufs=1) as cpool,
            tc.tile_pool(name="toks", bufs=6) as tpool,
            tc.tile_pool(name="prods", bufs=2) as ppool,
            tc.tile_pool(name="atws", bufs=2) as apool,
            tc.tile_pool(name="psum", bufs=1, space="PSUM") as pspool,
        ):
            # wcat rides the SWDGE (Pool) path so it never contends with the
            # token loads for HWDGE issue slots; it gates the weight
            # broadcasts which gate the whole DVE pipeline.
            with tc.high_priority():
                wcat_sb = cpool.tile([1, 2 * H + 1], F32)
                nc.scalar.dma_start(out=wcat_sb[:], in_=wcat[:])

            # token chunk loads own the SP HWDGE queue (~75% of all bytes)
            t0 = tpool.tile([P, H + MW], F32)
            nc.sync.dma_start(out=t0[:], in_=tok0[:])
            t1 = tpool.tile([P, H], F32)
            nc.sync.dma_start(out=t1[:], in_=tok1[:])
            t2 = tpool.tile([P, H], F32)
            nc.sync.dma_start(out=t2[:], in_=tok2[:])
            t3 = tpool.tile([P, H], F32)
            nc.sync.dma_start(out=t3[:], in_=tok3[:])
            meta_sb = t0[:, H:H + MW]

            # weight rows broadcast across partitions (GpSimd cross-partition
            # op; overlaps the token DMA stream)
            wb_src = cpool.tile([P, H], F32)
            wb_tgt = cpool.tile([P, H], F32)
            with tc.high_priority():
                nc.gpsimd.partition_broadcast(wb_src[:], wcat_sb[0:1, 0:H])
                nc.gpsimd.partition_broadcast(wb_tgt[:], wcat_sb[0:1, H:2 * H])

            # iota constant rides the idle ACT DGE queue
            iota_f = cpool.tile([P, P], F32)
            nc.scalar.dma_start(out=iota_f[:], in_=iotac[:])

            # bias column [S, 1]: broadcast b down the partitions (Pool,
            # off the critical path); added during the final copy-out
            bias_col = cpool.tile([P, 1], F32)
            nc.gpsimd.partition_broadcast(bias_col[:], wcat_sb[0:1, 2 * H:2 * H + 1])

            if mm_mode == "mat":
                ones_pt = cpool.tile([P, P], F32)
                nc.vector.memset(ones_pt[:], 1.0)

            psum_out = pspool.tile([S, T], F32)

            # per-chunk token pieces: (tile, column range) pairs
            chunk_pieces = [
                [(t0, 0, H)],
                [(t1, 0, H)],
                [(t2, 0, H)],
                [(t3, 0, H)],
            ]

            # membership tiles: atw_c[t, w] = (seg[t] == w) * wgt[t].
            # Built on DVE (GpSimd tensor_scalar is ~2.4us/op on HW), before
            # the reduce chain so they are off the critical tail.
            atw_tiles = []
            for c in range(NCHUNK):
                width = S if c < N_SRC_CHUNKS else T
                atw_c = apool.tile([P, P], F32, name=f"atw_{c}")
                nc.vector.tensor_scalar(
                    out=atw_c[:, :width], in0=iota_f[:, :width],
                    scalar1=meta_sb[:, 2 * c:2 * c + 1],
                    scalar2=meta_sb[:, 2 * c + 1:2 * c + 2],
                    op0=mybir.AluOpType.is_equal, op1=mybir.AluOpType.mult)
                atw_tiles.append(atw_c)

            u_sb = cpool.tile([P, 2 * NCHUNK], F32)
            scratch_col = NCHUNK
            for c in range(NCHUNK):
                is_src = c < N_SRC_CHUNKS
                width = S if is_src else T
                wb = wb_src if is_src else wb_tgt

                # u_c[t] = tok_c[t, :] . w  -- fused multiply+reduce on DVE
                # (AFFINE_MUL_REDUCE custom op; seed is 0 so multi-piece
                # chunks sum their partials with one [P,1] add)
                pieces = chunk_pieces[c]
                accs = []
                for pi, (tile_, j0, j1) in enumerate(pieces):
                    if len(pieces) == 1:
                        acc = u_sb[:, c:c + 1]
                    else:
                        acc = u_sb[:, scratch_col:scratch_col + 1]
                        scratch_col += 1
                    prod = ppool.tile([P, j1 - j0], F32, name=f"prod_{c}_{pi}",
                                      space=prod_space)
                    nc.vector.affine_mul_reduce(
                        out=prod[:], accum_out=acc, in0=tile_[:, 0:j1 - j0],
                        in1=wb[:, j0:j1], scale=1.0, bias=0.0)
                    accs.append(acc)
                if len(accs) > 1:
                    nc.vector.tensor_tensor(
                        out=u_sb[:, c:c + 1], in0=accs[0], in1=accs[1],
                        op=mybir.AluOpType.add)

                atw_c = atw_tiles[c]
                u_b = u_sb[:, c:c + 1]
                if mm_mode == "mat":
                    ub_mat = ppool.tile([P, P], F32, name=f"ubm_{c}", tag="ubm")
                    nc.vector.tensor_scalar_mul(ub_mat[:], ones_pt[:], u_b)
                    rhs_b, lhs_b = ub_mat[:, :T], ub_mat[:, :S]
                else:
                    rhs_b, lhs_b = u_b.broadcast_to([P, T]), u_b.broadcast_to([P, S])
                first = c == 0
                last = c == NCHUNK - 1
                if is_src:
                    # psum[s, t] += sum_t' atw[t', s] * u[t']  (same for all t)
                    nc.tensor.matmul(
                        psum_out[:], atw_c[:, :S], rhs_b,
                        start=first, stop=last)
                else:
                    nc.tensor.matmul(
                        psum_out[:], lhs_b, atw_c[:, :T],
                        start=first, stop=last)

            out_sb = cpool.tile([S, T], F32)
            nc.vector.tensor_scalar_add(out_sb[:], psum_out[:], bias_col[0:S, :])
            nc.sync.dma_start(out=out[:], in_=out_sb[:])


# ---------------------------------------------------------------------------
# Device kernel, general fallback: tokens may map into either word block
# ---------------------------------------------------------------------------

def _build_general(nc, S, T):
    NW = S + T
    tok = nc.declare_dram_parameter("tok", [L, H], F32, isOutput=False)
    atw = nc.declare_dram_parameter("atw", [NCHUNK, P, NW], F32, isOutput=False)
    wcat = nc.declare_dram_parameter("wcat", [1, 2 * H + 1], F32, isOutput=False)
    out = nc.declare_dram_parameter("out", [S, T], F32, isOutput=True)

    with TileContext(nc) as tc:
        with (
            tc.tile_pool(name="const", bufs=1) as cpool,
            tc.tile_pool(name="toks", bufs=3) as tpool,
            tc.tile_pool(name="prods", bufs=2) as ppool,
            tc.tile_pool(name="atws", bufs=2) as apool,
            tc.tile_pool(name="psum", bufs=1, space="PSUM") as pspool,
        ):
            wcat_sb = cpool.tile([1, 2 * H + 1], F32)
            nc.gpsimd.dma_start(out=wcat_sb[:], in_=wcat[:])
            ones = cpool.tile([1, P], F32)
            nc.vector.memset(ones[:], 1.0)
            bias_row = cpool.tile([1, T], F32)
            nc.vector.tensor_scalar_mul(
                bias_row[:], ones[:, :T], wcat_sb[0:1, 2 * H:2 * H + 1])

            wb_src = pspool.tile([P, H], F32)
            wb_tgt = pspool.tile([P, H], F32)
            for wb, w0 in ((wb_src, 0), (wb_tgt, H)):
                for j0, j1 in ((0, 512), (512, H)):
                    nc.tensor.matmul(
                        wb[:, j0:j1], ones[:, :P], wcat_sb[0:1, w0 + j0:w0 + j1],
                        start=True, stop=True)

            psum_out = pspool.tile([S, T], F32)
            nc.tensor.matmul(psum_out[:], ones[:, :S], bias_row[:],
                             start=True, stop=False)

            u_src_sb = cpool.tile([P, NCHUNK], F32)
            u_tgt_sb = cpool.tile([P, NCHUNK], F32)
            for c in range(NCHUNK):
                tok_c = tpool.tile([P, H], F32, name=f"tok_{c}")
                nc.sync.dma_start(out=tok_c[:], in_=tok[c * P:(c + 1) * P, :])
                for kind, wb, usb in (("s", wb_src, u_src_sb), ("t", wb_tgt, u_tgt_sb)):
                    prod = ppool.tile([P, H], F32, name=f"prod_{kind}_{c}")
                    nc.vector.affine_mul_reduce(
                        out=prod[:], accum_out=usb[:, c:c + 1], in0=tok_c[:],
                        in1=wb[:], scale=1.0, bias=0.0)

                atw_c = apool.tile([P, NW], F32, name=f"atw_{c}")
                nc.sync.dma_start(out=atw_c[:], in_=atw[c])
                last = c == NCHUNK - 1
                nc.tensor.matmul(
                    psum_out[:], atw_c[:, :S], u_src_sb[:, c:c + 1].broadcast_to([P, T]),
                    start=False, stop=False)
                nc.tensor.matmul(
                    psum_out[:], u_tgt_sb[:, c:c + 1].broadcast_to([P, S]), atw_c[:, S:],
                    start=False, stop=last)

            out_sb = cpool.tile([S, T], F32)
            nc.vector.tensor_scalar_add(out_sb[:], psum_out[:], bias_col[0:S, :])
            nc.sync.dma_start(out=out[:], in_=out_sb[:])


# variant knobs (fixed at import for the graded path; bench overrides)
MM_MODE = "mat"
PROD_SPACE = "SBUF"


@functools.lru_cache(maxsize=4)
def _build(S, T, block_ok, mm_mode=None, prod_space=None):
    mm_mode = MM_MODE if mm_mode is None else mm_mode
    prod_space = PROD_SPACE if prod_space is None else prod_space
    nc = bacc.Bacc("TRN2", debug=False, num_devices=N_CORES)
    if block_ok:
        prm = _declare_block_params(nc, S, T)
        with TileContext(nc) as tc:
            _emit_block_body(nc, tc, prm, S, T, mm_mode, prod_space)
    else:
        _build_general(nc, S, T)
    nc.compile()
    return nc


@functools.lru_cache(maxsize=16)
def _build_looped(S, T, iters, mm_mode=None, prod_space=None):
    """Timing-only variant: the same body repeated `iters` times inside one
    NEFF via a Tile For_i loop (per-iteration all-engine barrier back-edge)."""
    mm_mode = MM_MODE if mm_mode is None else mm_mode
    prod_space = PROD_SPACE if prod_space is None else prod_space
    nc = bacc.Bacc("TRN2", debug=False, num_devices=N_CORES)
    prm = _declare_block_params(nc, S, T)
    with TileContext(nc) as tc:
        with tc.For_i(0, iters, 1):
            _emit_block_body(nc, tc, prm, S, T, mm_mode, prod_space)
    nc.compile()
    return nc


# ---------------------------------------------------------------------------
# Host wrapper
# ---------------------------------------------------------------------------

def _prep(inputs):
    tok_h = np.ascontiguousarray(np.asarray(inputs["tok_h"], dtype=np.float32))
    mask = np.asarray(inputs["attention_mask"])
    swid = np.asarray(inputs["source_word_ids"])
    twid = np.asarray(inputs["target_word_ids"])
    W = np.asarray(inputs["W"], dtype=np.float32)
    b = np.asarray(inputs["b"], dtype=np.float32)
    S = int(np.asarray(inputs["S"]))
    T = int(np.asarray(inputs["T"]))

    Bv, Lv, Hv = tok_h.shape
    assert (Bv, Lv, Hv) == (B, L, H), f"unexpected tok_h shape {tok_h.shape}"
    assert swid.shape == (B, L_SRC) and twid.shape == (B, L_TGT)
    assert S <= P and T <= P

    NW = S + T
    combined = np.concatenate([swid, twid], axis=1).astype(np.int64)
    seg, valid = _segments(combined, mask, NW)
    wgt = _seg_weights(seg, valid, NW)

    src_tok_seg = seg[:, :L_SRC][valid[:, :L_SRC]]
    tgt_tok_seg = seg[:, L_SRC:][valid[:, L_SRC:]]
    block_ok = bool(
        (src_tok_seg < S).all()
        and (tgt_tok_seg >= S).all() and (tgt_tok_seg < NW).all()
    )

    wcat = np.zeros((1, 2 * H + 1), dtype=np.float32)
    wcat[0, :H] = W[:H, 0]
    wcat[0, H:2 * H] = W[H:2 * H, 0]
    wcat[0, 2 * H] = b.reshape(-1)[0]

    in_maps = []
    if block_ok:
        # meta[b, t_local, 2c] = in-block segment col (or -1), [.., 2c+1] = wgt
        meta = np.zeros((B, P, 2 * NCHUNK), dtype=np.float32)
        for bi in range(B):
            for c in range(NCHUNK):
                tsl = slice(c * P, (c + 1) * P)
                segc = seg[bi, tsl].astype(np.int64)
                col = segc if c < N_SRC_CHUNKS else segc - S
                ok = valid[bi, tsl] & (segc < NW)
                meta[bi, :, 2 * c] = np.where(ok, col, -1).astype(np.float32)
                meta[bi, :, 2 * c + 1] = wgt[bi, tsl]
        for i in range(N_CORES):
            bi = i % B
            tk = tok_h[bi]
            in_maps.append({
                # chunk 0 carries meta as extra columns
                "tok0": np.ascontiguousarray(
                    np.concatenate([tk[0:P, :], meta[bi]], axis=1)),
                "tok1": np.ascontiguousarray(tk[P:2 * P, :]),
                "tok2": np.ascontiguousarray(tk[2 * P:3 * P, :]),
                "tok3": np.ascontiguousarray(tk[3 * P:4 * P, :]),
                "wcat": wcat,
                "iotac": np.tile(np.arange(P, dtype=np.float32), (P, 1)),
            })
    else:
        atw = np.zeros((B, NCHUNK, P, NW), dtype=np.float32)
        for bi in range(B):
            for t in range(L):
                s = seg[bi, t]
                if s >= NW or not valid[bi, t]:
                    continue
                atw[bi, t // P, t % P, s] = wgt[bi, t]
        for i in range(N_CORES):
            bi = i % B
            in_maps.append({"tok": tok_h[bi], "atw": atw[bi], "wcat": wcat})
    return S, T, block_ok, in_maps


def kernel(**inputs):
    S, T, block_ok, in_maps = _prep(inputs)
    nc = _build(S, T, block_ok)
    res = run_bass_kernel_spmd(nc, in_maps, core_ids=list(range(N_CORES)))
    return np.stack([res.results[i]["out"] for i in range(B)], axis=0)


@functools.lru_cache(maxsize=4)
def _build_looped_empty(iters):
    """Calibration: same For_i loop with a minimal body, to measure the
    per-iteration loop overhead (back-edge barrier + sem reset)."""
    nc = bacc.Bacc("TRN2", debug=False, num_devices=N_CORES)
    x = nc.declare_dram_parameter("x", [P, 16], F32, isOutput=False)
    y = nc.declare_dram_parameter("y", [P, 16], F32, isOutput=True)
    with TileContext(nc) as tc:
        with tc.tile_pool(name="p", bufs=2) as pool:
            t = pool.tile([P, 16], F32)
            nc.sync.dma_start(out=t[:], in_=x[:])
            with tc.For_i(0, iters, 1):
                w = pool.tile([P, 16], F32)
                nc.vector.tensor_copy(w[:], t[:])
            nc.sync.dma_start(out=y[:], in_=t[:])
    nc.compile()
    return nc



# revision 21
# speedup vs baseline: 1.3338x; 1.3338x over previous
"""Trainium2 Bass kernel for nn_BinaryTokenClassificationModel (segment_reduce).

Math: the reference mean-pools token embeddings into word embeddings over
contiguous runs of equal word ids, then computes
    logits[b,s,t] = src_pooled[b,s] @ w_src + tgt_pooled[b,t] @ w_tgt + b.
Pooling and the linear classifier commute, so with the host-precomputed
weighted membership matrix  atw[tok, word] = (seg[tok]==word) / count(word)
each core (batch row) computes
    u[tok]      = tok_h[tok, :] . w                (DVE fused multiply-reduce)
    psum[s, t] += atw_src^T @ bcast(u_src)         (TensorE, per src chunk)
    psum[s, t] += bcast(u_tgt) @ atw_tgt           (TensorE, per tgt chunk)
    out         = psum + bias                      (DVE) -> DMA out
Implementation notes (raw bass, no Tile framework):
  - token data, membership and broadcast weights are uploaded in bf16
    (halves HBM traffic; fp32 accumulation throughout keeps rel err ~1e-3)
  - per-chunk columns pack tok_h and atw into one DMA; w_src/w_tgt rows are
    pre-broadcast on the host so no on-device partition broadcast is needed
  - manual semaphores; only ~5 sems allocated, cleared with one range-clear
    (the Tile framework's ~250-sem end-of-kernel sweep costs ~6 us)
Data-parallel over batch: core i handles batch row i. No collectives.
"""

import functools
from contextlib import ExitStack

import numpy as np
import ml_dtypes

import concourse.bacc as bacc
import concourse.mybir as mybir
from concourse.bass_utils import run_bass_kernel_spmd

# Problem geometry (hardcoded per spec)
B = 8
L_SRC = 256
L_TGT = 256
L = L_SRC + L_TGT  # 512
H = 768
P = 128            # SBUF partitions / tokens per chunk
NCHUNK = L // P    # 4
N_SRC_CHUNKS = L_SRC // P  # 2
N_CORES = 8
F32 = mybir.dt.float32
BF16 = mybir.dt.bfloat16
NPBF16 = ml_dtypes.bfloat16


# ---------------------------------------------------------------------------
# Host-side segment bookkeeping (exact mirror of reference._pool_words)
# ---------------------------------------------------------------------------

def _segments(combined_wid, attention_mask, n_words):
    """Per-token dense run ids exactly as the reference computes them."""
    valid = (attention_mask > 0) & (combined_wid >= 0)  # [B, L]
    prev_wid = np.concatenate(
        [np.full((combined_wid.shape[0], 1), -2, dtype=combined_wid.dtype),
         combined_wid[:, :-1]], axis=1)
    prev_valid = np.concatenate(
        [np.zeros((valid.shape[0], 1), dtype=bool), valid[:, :-1]], axis=1)
    new_run = valid & ((combined_wid != prev_wid) | (~prev_valid))
    run_id = np.cumsum(new_run.astype(np.int64), axis=1) - 1  # [B, L]
    seg = np.where(valid, run_id, n_words)  # n_words = dummy slot
    return seg, valid


def _seg_weights(seg, valid, n_words):
    """1/max(count,1) weight for each token's segment (0 for invalid)."""
    Bv, Lv = seg.shape
    wgt = np.zeros((Bv, Lv), dtype=np.float32)
    for b in range(Bv):
        counts = np.bincount(seg[b][valid[b]], minlength=Lv + 1).astype(np.float32)
        inv = 1.0 / np.maximum(counts, 1.0)
        wgt[b] = np.where(valid[b] & (seg[b] < n_words), inv[np.minimum(seg[b], Lv)], 0.0)
    return wgt


# ---------------------------------------------------------------------------
# Device kernel (raw bass)
# ---------------------------------------------------------------------------

def _chunk_layout(S, T, block_ok):
    """Per-chunk packed column layout: tok_h (H cols) then atw columns.

    block_ok: src chunks carry S membership cols, tgt chunks T cols.
    general:  every chunk carries S+T cols (src block then tgt block).
    Returns (offsets, widths, atw_meta) where atw_meta[c] = (src_off, tgt_off)
    with None when that block is absent.
    """
    offs, widths, meta = [], [], []
    pos = 0
    for c in range(NCHUNK):
        is_src = c < N_SRC_CHUNKS
        if block_ok:
            wa = S if is_src else T
            meta.append((pos + H, None) if is_src else (None, pos + H))
        else:
            wa = S + T
            meta.append((pos + H, pos + H + S))
        offs.append(pos)
        widths.append(H + wa)
        pos += H + wa
    return offs, widths, meta, pos


@functools.lru_cache(maxsize=4)
def _build(S, T, block_ok):
    nc = bacc.Bacc("TRN2", debug=False, num_devices=N_CORES)
    offs, widths, meta, totw = _chunk_layout(S, T, block_ok)

    tok = nc.declare_dram_parameter("tok", [P, totw], BF16, isOutput=False)
    # wbc = [w_src bcast (H) | w_tgt bcast (H)]
    wbc = nc.declare_dram_parameter("wbc", [P, 2 * H], BF16, isOutput=False)
    # bias broadcast down the partitions (f32: tensor_scalar add needs f32 AP)
    biascol = nc.declare_dram_parameter("biascol", [P, 1], F32, isOutput=False)
    out = nc.declare_dram_parameter("out", [S, T], F32, isOutput=True)

    with ExitStack() as ctx:
        tok_sb = ctx.enter_context(nc.sbuf_tensor([P, totw], BF16))
        wbc_sb = ctx.enter_context(nc.sbuf_tensor([P, 2 * H], BF16))
        bias_sb = ctx.enter_context(nc.sbuf_tensor([P, 1], F32))
        prod = ctx.enter_context(nc.sbuf_tensor([P, H], BF16))
        n_mm = NCHUNK if block_ok else 2 * NCHUNK
        ub = ctx.enter_context(nc.sbuf_tensor([P, n_mm * P], BF16))
        u = ctx.enter_context(nc.sbuf_tensor([P, n_mm], F32))
        out_sb = ctx.enter_context(nc.sbuf_tensor([S, T], F32))
        psum = ctx.enter_context(nc.psum_tensor([S, T], F32))

        # one sem per DMA: a shared queue sem's 16 per-engine increments
        # interleave across transfers, so intermediate counts are unordered
        ck = [ctx.enter_context(nc.semaphore(name=f"ck{c}"))
              for c in range(NCHUNK)]
        wsrc_sem = ctx.enter_context(nc.semaphore())
        wtgt_sem = ctx.enter_context(nc.semaphore())
        bias_sem = ctx.enter_context(nc.semaphore())
        outd_sem = ctx.enter_context(nc.semaphore())
        u_sem = ctx.enter_context(nc.semaphore())
        ub_sem = ctx.enter_context(nc.semaphore())
        mm_sem = ctx.enter_context(nc.semaphore())
        add_sem = ctx.enter_context(nc.semaphore())
        sems = ck + [wsrc_sem, wtgt_sem, bias_sem, outd_sem,
                     u_sem, ub_sem, mm_sem, add_sem]
        sem_nums = sorted(s.num for s in sems)
        assert sem_nums[-1] - sem_nums[0] == len(sems) - 1, sem_nums

        # per-chunk matmul plan: (c, kind) in chain order
        mm_plan = []
        for c in range(NCHUNK):
            src_off, tgt_off = meta[c]
            if src_off is not None:
                mm_plan.append((c, "src", src_off))
            if tgt_off is not None:
                mm_plan.append((c, "tgt", tgt_off))

        with nc.Block(no_gpsimd_drain=True) as block:

            @block.sync
            def _(sync):
                for c in range(NCHUNK):
                    sl = slice(offs[c], offs[c] + widths[c])
                    sync.dma_start(out=tok_sb[:, sl], in_=tok[:, sl]).then_inc(
                        ck[c], 16)
                sync.wait_ge(outd_sem, 16)

            @block.scalar
            def _(scalar):
                # w_src half first: the first (src) chunk's reduce needs it
                scalar.dma_start(out=wbc_sb[:, 0:H], in_=wbc[:, 0:H]).then_inc(
                    wsrc_sem, 16)
                scalar.dma_start(
                    out=wbc_sb[:, H:2 * H], in_=wbc[:, H:2 * H]
                ).then_inc(wtgt_sem, 16)
                scalar.dma_start(out=bias_sb[:, :], in_=biascol[:, :]).then_inc(
                    bias_sem, 16)
                scalar.wait_ge(add_sem, 1)
                scalar.dma_start(out=out[:, :], in_=out_sb[:, :]).then_inc(
                    outd_sem, 16)

            @block.vector
            def _(vector):
                seen_c = -1
                for i, (c, kind, _aoff) in enumerate(mm_plan):
                    if c != seen_c:
                        vector.wait_ge(ck[c], 16)
                        seen_c = c
                    vector.wait_ge(wsrc_sem if kind == "src" else wtgt_sem, 16)
                    woff = 0 if kind == "src" else H
                    ucol = u[:, i:i + 1]
                    nc.vector.affine_mul_reduce(
                        out=prod[:, :],
                        accum_out=ucol,
                        in0=tok_sb[:, offs[c]:offs[c] + H],
                        in1=wbc_sb[:, woff:woff + H],
                        scale=1.0, bias=0.0).then_inc(u_sem, 1)
                    # custom-op accum flush: the race detector wants an
                    # explicit sync even same-engine (HW drains anyway)
                    vector.wait_ge(u_sem, i + 1)
                    # ub[p, :] = u[p]  (in0 * 0 + u); in0 = landed tok cols
                    nc.vector.tensor_scalar(
                        out=ub[:, i * P:i * P + max(S, T)],
                        in0=tok_sb[:, offs[c]:offs[c] + max(S, T)],
                        scalar1=0.0, scalar2=ucol,
                        op0=mybir.AluOpType.mult,
                        op1=mybir.AluOpType.add,
                    ).then_inc(ub_sem, 1)
                # out = psum + bias
                vector.wait_ge(mm_sem, 1)
                vector.wait_ge(bias_sem, 16)
                nc.vector.tensor_scalar_add(
                    out=out_sb[:, :], in0=psum[:, :],
                    scalar1=bias_sb[0:S, :],
                ).then_inc(add_sem, 1)

            @block.tensor
            def _(tensor):
                # ub results arrive in mm_plan order (same emission order)
                for i, (c, kind, aoff) in enumerate(mm_plan):
                    tensor.wait_ge(ub_sem, i + 1)
                    first, last = i == 0, i == len(mm_plan) - 1
                    if kind == "src":
                        mm = nc.tensor.matmul(
                            psum[:, :],
                            tok_sb[:, aoff:aoff + S],
                            ub[:, i * P:i * P + T],
                            start=first, stop=last)
                    else:
                        mm = nc.tensor.matmul(
                            psum[:, :],
                            ub[:, i * P:i * P + S],
                            tok_sb[:, aoff:aoff + T],
                            start=first, stop=last)
                    if last:
                        mm.then_inc(mm_sem, 1)

        # after the Block's exit barrier every engine has synced on all sem
        # updates, so a single range-clear resets them for re-execution
        nc.sync.sem_clear(range(sem_nums[0], sem_nums[-1] + 1))

        nc.compile()
    return nc


# ---------------------------------------------------------------------------
# Host wrapper
# ---------------------------------------------------------------------------

def _prep(inputs):
    tok_h = np.ascontiguousarray(np.asarray(inputs["tok_h"], dtype=np.float32))
    mask = np.asarray(inputs["attention_mask"])
    swid = np.asarray(inputs["source_word_ids"])
    twid = np.asarray(inputs["target_word_ids"])
    W = np.asarray(inputs["W"], dtype=np.float32)
    b = np.asarray(inputs["b"], dtype=np.float32)
    S = int(np.asarray(inputs["S"]))
    T = int(np.asarray(inputs["T"]))

    Bv, Lv, Hv = tok_h.shape
    assert (Bv, Lv, Hv) == (B, L, H), f"unexpected tok_h shape {tok_h.shape}"
    assert swid.shape == (B, L_SRC) and twid.shape == (B, L_TGT)
    assert S <= P and T <= P

    NW = S + T
    combined = np.concatenate([swid, twid], axis=1).astype(np.int64)
    seg, valid = _segments(combined, mask, NW)
    wgt = _seg_weights(seg, valid, NW)

    src_tok_seg = seg[:, :L_SRC][valid[:, :L_SRC]]
    tgt_tok_seg = seg[:, L_SRC:][valid[:, L_SRC:]]
    block_ok = bool(
        (src_tok_seg < S).all()
        and (tgt_tok_seg >= S).all() and (tgt_tok_seg < NW).all()
    )

    offs, widths, meta, totw = _chunk_layout(S, T, block_ok)

    # wbc = [w_src bcast | w_tgt bcast]
    wbc = np.empty((P, 2 * H), dtype=NPBF16)
    wbc[:, 0:H] = np.broadcast_to(W[:H, 0], (P, H)).astype(NPBF16)
    wbc[:, H:2 * H] = np.broadcast_to(W[H:2 * H, 0], (P, H)).astype(NPBF16)
    biascol = np.full((P, 1), b.reshape(-1)[0], dtype=np.float32)

    tok_bf = tok_h.astype(NPBF16)
    in_maps = []
    for i in range(N_CORES):
        bi = i % B
        tokpack = np.zeros((P, totw), dtype=NPBF16)
        for c in range(NCHUNK):
            tsl = slice(c * P, (c + 1) * P)
            tokpack[:, offs[c]:offs[c] + H] = tok_bf[bi, tsl, :]
            segc = seg[bi, tsl]
            wgtc = wgt[bi, tsl]
            src_off, tgt_off = meta[c]
            # atw[tok, word] = wgt * (seg == word), split by block
            if src_off is not None:
                atw = np.zeros((P, S), dtype=np.float32)
                ok = segc < S
                atw[np.arange(P)[ok], segc[ok]] = wgtc[ok]
                tokpack[:, src_off:src_off + S] = atw.astype(NPBF16)
            if tgt_off is not None:
                atw = np.zeros((P, T), dtype=np.float32)
                ok = (segc >= S) & (segc < NW)
                atw[np.arange(P)[ok], segc[ok] - S] = wgtc[ok]
                tokpack[:, tgt_off:tgt_off + T] = atw.astype(NPBF16)
        in_maps.append({"tok": tokpack, "wbc": wbc, "biascol": biascol})
    return S, T, block_ok, in_maps


def kernel(**inputs):
    S, T, block_ok, in_maps = _prep(inputs)
    nc = _build(S, T, block_ok)
    res = run_bass_kernel_spmd(nc, in_maps, core_ids=list(range(N_CORES)))
    return np.stack([res.results[i]["out"] for i in range(B)], axis=0)


# revision 25
# speedup vs baseline: 1.4700x; 1.1021x over previous
"""Trainium2 Bass kernel for nn_BinaryTokenClassificationModel (segment_reduce).

Math: the reference mean-pools token embeddings into word embeddings over
contiguous runs of equal word ids, then computes
    logits[b,s,t] = src_pooled[b,s] @ w_src + tgt_pooled[b,t] @ w_tgt + b.
Pooling and the linear classifier commute, so with the host-precomputed
weighted membership matrix  atw[tok, word] = (seg[tok]==word) / count(word)
each core (batch row) computes
    u[tok]      = tok_h[tok, :] . w                (DVE fused multiply-reduce)
    psum[s, t] += atw_src^T @ bcast(u_src)         (TensorE, per src chunk)
    psum[s, t] += bcast(u_tgt) @ atw_tgt           (TensorE, per tgt chunk)
    out         = psum (+ bias via an extra rank-1 matmul in the chain)
Implementation notes (raw bass, no Tile framework):
  - token data and membership are uploaded in bf16, packed per 128-token
    chunk into one DMA each (fp32 accumulation keeps rel err ~3e-3)
  - w_src/w_tgt are one 3 KB row; TensorE broadcasts them down the 128
    partitions (ones-column matmul into PSUM) and the DVE reduce reads the
    broadcast weights directly from PSUM -- no partition_broadcast ucode,
    no 384 KB host-pre-broadcast upload
  - bias enters as the first matmul of the accumulation group
    (ones[1,S]^T @ bias_row[1,T]), so the epilogue is a plain ACT copy
  - manual semaphores (~12), cleared with one 36 ns range-clear; the Tile
    framework's end-of-kernel machinery costs ~8 us on this kernel
Data-parallel over batch: core i handles batch row i. No collectives.
"""

import functools
from contextlib import ExitStack

import numpy as np
import ml_dtypes

import concourse.bacc as bacc
import concourse.mybir as mybir
from concourse.bass_utils import run_bass_kernel_spmd

# Problem geometry (hardcoded per spec)
B = 8
L_SRC = 256
L_TGT = 256
L = L_SRC + L_TGT  # 512
H = 768
P = 128            # SBUF partitions / tokens per chunk
NCHUNK = L // P    # 4
N_SRC_CHUNKS = L_SRC // P  # 2
N_CORES = 8
F32 = mybir.dt.float32
BF16 = mybir.dt.bfloat16
NPBF16 = ml_dtypes.bfloat16


# ---------------------------------------------------------------------------
# Host-side segment bookkeeping (exact mirror of reference._pool_words)
# ---------------------------------------------------------------------------

def _segments(combined_wid, attention_mask, n_words):
    """Per-token dense run ids exactly as the reference computes them."""
    valid = (attention_mask > 0) & (combined_wid >= 0)  # [B, L]
    prev_wid = np.concatenate(
        [np.full((combined_wid.shape[0], 1), -2, dtype=combined_wid.dtype),
         combined_wid[:, :-1]], axis=1)
    prev_valid = np.concatenate(
        [np.zeros((valid.shape[0], 1), dtype=bool), valid[:, :-1]], axis=1)
    new_run = valid & ((combined_wid != prev_wid) | (~prev_valid))
    run_id = np.cumsum(new_run.astype(np.int64), axis=1) - 1  # [B, L]
    seg = np.where(valid, run_id, n_words)  # n_words = dummy slot
    return seg, valid


def _seg_weights(seg, valid, n_words):
    """1/max(count,1) weight for each token's segment (0 for invalid)."""
    Bv, Lv = seg.shape
    wgt = np.zeros((Bv, Lv), dtype=np.float32)
    for b in range(Bv):
        counts = np.bincount(seg[b][valid[b]], minlength=Lv + 1).astype(np.float32)
        inv = 1.0 / np.maximum(counts, 1.0)
        wgt[b] = np.where(valid[b] & (seg[b] < n_words), inv[np.minimum(seg[b], Lv)], 0.0)
    return wgt


# ---------------------------------------------------------------------------
# Device kernel (raw bass)
# ---------------------------------------------------------------------------

def _chunk_layout(S, T, block_ok):
    """Per-chunk packed column layout: tok_h (H cols) then atw columns.

    block_ok: src chunks carry S membership cols, tgt chunks T cols.
    general:  every chunk carries S+T cols (src block then tgt block).
    Returns (offsets, widths, atw_meta, total) with atw_meta[c] =
    (src_off, tgt_off), None when that block is absent.
    """
    offs, widths, meta = [], [], []
    pos = 0
    for c in range(NCHUNK):
        is_src = c < N_SRC_CHUNKS
        if block_ok:
            wa = S if is_src else T
            meta.append((pos + H, None) if is_src else (None, pos + H))
        else:
            wa = S + T
            meta.append((pos + H, pos + H + S))
        offs.append(pos)
        widths.append(H + wa)
        pos += H + wa
    return offs, widths, meta, pos


@functools.lru_cache(maxsize=4)
def _build(S, T, block_ok):
    nc = bacc.Bacc("TRN2", debug=False, num_devices=N_CORES)
    offs, widths, meta, totw = _chunk_layout(S, T, block_ok)

    tok = nc.declare_dram_parameter("tok", [P, totw], BF16, isOutput=False)
    # wcat = [w_src (H) | w_tgt (H) | bias row (T copies of b)]
    wcat = nc.declare_dram_parameter("wcat", [1, 2 * H + T], BF16, isOutput=False)
    out = nc.declare_dram_parameter("out", [S, T], F32, isOutput=True)

    with ExitStack() as ctx:
        tok_sb = ctx.enter_context(nc.sbuf_tensor([P, totw], BF16))
        wcat_sb = ctx.enter_context(nc.sbuf_tensor([1, 2 * H + T], BF16))
        ones = ctx.enter_context(nc.sbuf_tensor([1, P], BF16))
        prod = ctx.enter_context(nc.sbuf_tensor([P, H], BF16))
        n_mm = NCHUNK if block_ok else 2 * NCHUNK
        ub = ctx.enter_context(nc.sbuf_tensor([P, n_mm * P], BF16))
        u = ctx.enter_context(nc.sbuf_tensor([P, n_mm], F32))
        out_sb = ctx.enter_context(nc.sbuf_tensor([S, T], F32))
        # PSUM: broadcast weights (2 banks each) + the output accumulator
        wsrc_ps = ctx.enter_context(nc.psum_tensor([P, H], F32))
        wtgt_ps = ctx.enter_context(nc.psum_tensor([P, H], F32))
        psum = ctx.enter_context(nc.psum_tensor([S, T], F32))

        ck = [ctx.enter_context(nc.semaphore(name=f"ck{c}"))
              for c in range(NCHUNK)]
        wcat_sem = ctx.enter_context(nc.semaphore())
        outd_sem = ctx.enter_context(nc.semaphore())
        ones_sem = ctx.enter_context(nc.semaphore())
        wb_sem = ctx.enter_context(nc.semaphore())
        u_sem = ctx.enter_context(nc.semaphore())
        ub_sem = ctx.enter_context(nc.semaphore())
        mm_sem = ctx.enter_context(nc.semaphore())
        cp_sem = ctx.enter_context(nc.semaphore())
        sems = ck + [wcat_sem, outd_sem, ones_sem, wb_sem,
                     u_sem, ub_sem, mm_sem, cp_sem]
        sem_nums = sorted(s.num for s in sems)
        assert sem_nums[-1] - sem_nums[0] == len(sems) - 1, sem_nums

        # per-chunk matmul plan: (c, kind, atw col offset) in chain order
        mm_plan = []
        for c in range(NCHUNK):
            src_off, tgt_off = meta[c]
            if src_off is not None:
                mm_plan.append((c, "src", src_off))
            if tgt_off is not None:
                mm_plan.append((c, "tgt", tgt_off))

        with nc.Block(no_gpsimd_drain=True) as block:

            @block.sync
            def _(sync):
                # weights row first (tiny; everything upstream needs it)
                sync.dma_start(out=wcat_sb[:, :], in_=wcat[:, :]).then_inc(
                    wcat_sem, 16)
                for c in range(NCHUNK):
                    sl = slice(offs[c], offs[c] + widths[c])
                    sync.dma_start(out=tok_sb[:, sl], in_=tok[:, sl]).then_inc(
                        ck[c], 16)
                sync.wait_ge(outd_sem, 16)

            @block.vector
            def _(vector):
                nc.vector.memset(ones[:, :], 1.0).then_inc(ones_sem, 1)
                seen_c = -1
                for i, (c, kind, _aoff) in enumerate(mm_plan):
                    if c != seen_c:
                        vector.wait_ge(ck[c], 16)
                        seen_c = c
                    vector.wait_ge(wb_sem, 1 if kind == "src" else 2)
                    wps = wsrc_ps if kind == "src" else wtgt_ps
                    ucol = u[:, i:i + 1]
                    nc.vector.affine_mul_reduce(
                        out=prod[:, :],
                        accum_out=ucol,
                        in0=tok_sb[:, offs[c]:offs[c] + H],
                        in1=wps[:, :],
                        scale=1.0, bias=0.0).then_inc(u_sem, 1)
                    # custom-op accum flush: the race detector wants explicit
                    # sync even same-engine (HW drains anyway)
                    vector.wait_ge(u_sem, i + 1)
                    # ub[p, :] = u[p]  (in0 * 0 + u); in0 = landed tok cols
                    nc.vector.tensor_scalar(
                        out=ub[:, i * P:i * P + max(S, T)],
                        in0=tok_sb[:, offs[c]:offs[c] + max(S, T)],
                        scalar1=0.0, scalar2=ucol,
                        op0=mybir.AluOpType.mult,
                        op1=mybir.AluOpType.add,
                    ).then_inc(ub_sem, 1)

            @block.tensor
            def _(tensor):
                tensor.wait_ge(wcat_sem, 16)
                tensor.wait_ge(ones_sem, 1)
                # broadcast w rows down the partitions: [P,H] = ones^T @ w_row
                for wi, wps in ((0, wsrc_ps), (1, wtgt_ps)):
                    for j0, j1 in ((0, 512), (512, H)):
                        mm = nc.tensor.matmul(
                            wps[:, j0:j1], ones[:, :P],
                            wcat_sb[:, wi * H + j0:wi * H + j1],
                            start=True, stop=True)
                        if j1 == H:
                            mm.then_inc(wb_sem, 1)
                # bias enters the output accumulation group first
                nc.tensor.matmul(
                    psum[:, :], ones[:, :S], wcat_sb[:, 2 * H:2 * H + T],
                    start=True, stop=False)
                for i, (c, kind, aoff) in enumerate(mm_plan):
                    tensor.wait_ge(ub_sem, i + 1)
                    last = i == len(mm_plan) - 1
                    if kind == "src":
                        mm = nc.tensor.matmul(
                            psum[:, :],
                            tok_sb[:, aoff:aoff + S],
                            ub[:, i * P:i * P + T],
                            start=False, stop=last)
                    else:
                        mm = nc.tensor.matmul(
                            psum[:, :],
                            ub[:, i * P:i * P + S],
                            tok_sb[:, aoff:aoff + T],
                            start=False, stop=last)
                    if last:
                        mm.then_inc(mm_sem, 1)

            @block.scalar
            def _(scalar):
                scalar.wait_ge(mm_sem, 1)
                nc.scalar.copy(out_sb[:, :], psum[:, :]).then_inc(cp_sem, 1)
                scalar.wait_ge(cp_sem, 1)
                scalar.dma_start(out=out[:, :], in_=out_sb[:, :]).then_inc(
                    outd_sem, 16)

        # after the Block's exit barrier every engine has synced on all sem
        # updates, so a single range-clear resets them for re-execution
        nc.sync.sem_clear(range(sem_nums[0], sem_nums[-1] + 1))

        nc.compile()
    return nc


# ---------------------------------------------------------------------------
# Host wrapper
# ---------------------------------------------------------------------------

def _prep(inputs):
    tok_h = np.ascontiguousarray(np.asarray(inputs["tok_h"], dtype=np.float32))
    mask = np.asarray(inputs["attention_mask"])
    swid = np.asarray(inputs["source_word_ids"])
    twid = np.asarray(inputs["target_word_ids"])
    W = np.asarray(inputs["W"], dtype=np.float32)
    b = np.asarray(inputs["b"], dtype=np.float32)
    S = int(np.asarray(inputs["S"]))
    T = int(np.asarray(inputs["T"]))

    Bv, Lv, Hv = tok_h.shape
    assert (Bv, Lv, Hv) == (B, L, H), f"unexpected tok_h shape {tok_h.shape}"
    assert swid.shape == (B, L_SRC) and twid.shape == (B, L_TGT)
    assert S <= P and T <= P

    NW = S + T
    combined = np.concatenate([swid, twid], axis=1).astype(np.int64)
    seg, valid = _segments(combined, mask, NW)
    wgt = _seg_weights(seg, valid, NW)

    src_tok_seg = seg[:, :L_SRC][valid[:, :L_SRC]]
    tgt_tok_seg = seg[:, L_SRC:][valid[:, L_SRC:]]
    block_ok = bool(
        (src_tok_seg < S).all()
        and (tgt_tok_seg >= S).all() and (tgt_tok_seg < NW).all()
    )

    offs, widths, meta, totw = _chunk_layout(S, T, block_ok)

    # wcat row: [w_src | w_tgt | bias row]
    wcat = np.empty((1, 2 * H + T), dtype=NPBF16)
    wcat[0, 0:H] = W[:H, 0].astype(NPBF16)
    wcat[0, H:2 * H] = W[H:2 * H, 0].astype(NPBF16)
    wcat[0, 2 * H:] = NPBF16(b.reshape(-1)[0])

    tok_bf = tok_h.astype(NPBF16)
    in_maps = []
    for i in range(N_CORES):
        bi = i % B
        tokpack = np.zeros((P, totw), dtype=NPBF16)
        for c in range(NCHUNK):
            tsl = slice(c * P, (c + 1) * P)
            tokpack[:, offs[c]:offs[c] + H] = tok_bf[bi, tsl, :]
            segc = seg[bi, tsl]
            wgtc = wgt[bi, tsl]
            src_off, tgt_off = meta[c]
            # atw[tok, word] = wgt * (seg == word), split by block
            if src_off is not None:
                atw = np.zeros((P, S), dtype=np.float32)
                ok = segc < S
                atw[np.arange(P)[ok], segc[ok]] = wgtc[ok]
                tokpack[:, src_off:src_off + S] = atw.astype(NPBF16)
            if tgt_off is not None:
                atw = np.zeros((P, T), dtype=np.float32)
                ok = (segc >= S) & (segc < NW)
                atw[np.arange(P)[ok], segc[ok] - S] = wgtc[ok]
                tokpack[:, tgt_off:tgt_off + T] = atw.astype(NPBF16)
        in_maps.append({"tok": tokpack, "wcat": wcat})
    return S, T, block_ok, in_maps


def kernel(**inputs):
    S, T, block_ok, in_maps = _prep(inputs)
    nc = _build(S, T, block_ok)
    res = run_bass_kernel_spmd(nc, in_maps, core_ids=list(range(N_CORES)))
    return np.stack([res.results[i]["out"] for i in range(B)], axis=0)
